# revision 1
# baseline (speedup 1.0000x reference)
"""Trainium2 Bass kernel for 8-head MultiHeadAttention (B=4, S=2048, D=512).

Sharding: tensor-parallel over heads -- core c owns head c. All matmul
operands are float32r (full fp32 bytes, 1 cycle/row on the PE at moving
dim >= 256; ~10x more accurate than bf16 as measured on HW). Each core:
  K^T,V^T = Wkv_h @ x^T    (packed KV projection, d-chunked matmuls; V^T is
                            PE-transposed per 512-token group into V' tiles
                            [128, 65] whose last column is ones)
  Q^T     = (Wq_h/8) @ q^T
  logits^T[k,q] = K^T.T @ Q^T per (batch, k-chunk)   [contraction Dh=64]
  expT = exp(logits^T)     (ScalarE, [128,1024] tiles; no max subtraction --
                            logits ~ N(0,1), fp32-safe)
  outT'[m,q] = sum_k V'[k,m] expT[k,q]   (row 64 accumulates sumexp free)
  y_partial = (outT / sumexp) @ Wo_h^T   (divide folded into a per-partition
                            scale of the 128-token output tiles)
Host sums the 8 partial y's and adds bo.

Software pipelining: emission order is proj(0); per batch b {attention qt0,
sums qt0, proj(b+1), attention qt1, sums qt1, y(b-1)}; y(B-1) last. The Tile
scheduler turns this into proj/y work riding under the ACT-bound attention.
PSUM: pa 2x[128,512] (proj/vtrans/y) + lp 2x[128,1024] (logits) +
op 1x[65,1024] (attnV accum) = 8 banks exactly.
Measured on HW: rel err 3.25e-4; cost-model 203.8 us/core (PE 158 busy,
DMA 146, ACT 135).
"""

import numpy as np

import concourse.bass as bass
import concourse.mybir as mybir
from concourse.tile import TileContext
from concourse.bass_utils import run_bass_kernel_spmd

# ---------------------------------------------------------------------------
# Workaround: this container's walrus rejects >1 sync wait on an InstDrain
# (TPB_CTRL). Split the TileContext exit-drain waits across single-wait NOPs.
_PATCHED = False


def _install_drain_patch():
    global _PATCHED
    if _PATCHED:
        return
    from concourse.vector_clock import ScopedClock, VectorClock

    def _split_drain_and_barrier(self, tick_clock, wait_clock):
        g = tick_clock.global_clock
        n = len(g)
        for i in range(n):
            t = g[i]
            if t > 0:
                vec = [0] * n
                vec[i] = t
                nop = self.nc.sync.nop(nofuse=True, hint=f"drain_wait_p{i}")
                wait_clock.add_sem_waits(
                    nop.ins, ScopedClock({None: VectorClock(vec)})
                )
        self.nc.sync.drain()
        self.nc.all_engine_barrier()
        assert self.sems is not None
        popped = self.nc._tile_sem_poison_stack.pop()
        assert popped is self._sem_poison
        self.nc.clear_and_free_semaphores(list(self.sems.allocated().values()))
        self.nc.all_engine_barrier()

    TileContext._drain_and_barrier = _split_drain_and_barrier
    _PATCHED = True


def _split_multi_waits(nc):
    """This walrus accepts at most ONE sync wait per instruction. Hoist extra
    waits onto same-engine NOPs inserted immediately before the instruction
    (same-engine program order preserves semantics)."""
    n_split = 0
    for blk in nc.m.functions[0].blocks:
        il = blk.instructions
        i = 0
        while i < len(il):
            inst = il[i]
            try:
                si = inst.sync_info
            except AttributeError:
                si = None
            if si is not None and si.on_wait is not None and len(si.on_wait) > 1:
                waits = list(si.on_wait)
                for j, w in enumerate(waits[:-1]):
                    nop = mybir.InstNoOp(
                        name=f"{inst.name}_hw{j}",
                        sync_info=mybir.SyncInfo(on_wait=[w], on_update=[]),
                        bass_nofuse=True,
                        engine=inst.engine,
                    )
                    il.insert(i, nop)
                    i += 1
                inst.sync_info = mybir.SyncInfo(
                    on_wait=[waits[-1]], on_update=list(si.on_update)
                )
                n_split += 1
            i += 1
    return n_split


# ---------------------------------------------------------------------------
B, S, D, H = 4, 2048, 512, 8
Dh = D // H  # 64
T = B * S  # 8192
NCORES = 8

F32 = mybir.dt.float32
F32R = mybir.dt.float32r
BF16 = mybir.dt.bfloat16
NP_BF16 = mybir.dt.np(BF16)

TT = 512  # projection token tile
NTT = T // TT  # 16
QTILE = 1024  # q tile for logits/exp
KC = 128  # k chunk (PSUM partitions)
NKT = T // KC  # 64 global k tiles
VW = Dh + 1  # V' width (ones column appended)



def _evac_bias(nc, out_ap, in_ap, bias_ap, on_act):
    """PSUM->SBUF evacuation with per-partition bias add, on ACT or DVE."""
    if on_act:
        nc.scalar.activation(
            out_ap, in_ap, mybir.ActivationFunctionType.Identity, bias=bias_ap
        )
    else:
        nc.vector.tensor_scalar_add(out_ap, in_ap, bias_ap)


def _build(reps: int = 1, loop_n: int = 0) -> bass.Bass:
    nc = bass.Bass(name="mha")
    xT = nc.dram_tensor("xT", [4, 128, T], F32R, kind="ExternalInput")
    qT = nc.dram_tensor("qT", [4, 128, T], F32R, kind="ExternalInput")
    wkv = nc.dram_tensor("wkv", [4, 128, 2 * Dh], F32R, kind="ExternalInput")
    bkv = nc.dram_tensor("bkv", [128, 1], F32, kind="ExternalInput")
    wq = nc.dram_tensor("wq", [4, 128, Dh], F32R, kind="ExternalInput")
    bq = nc.dram_tensor("bq", [Dh, 1], F32, kind="ExternalInput")
    wo = nc.dram_tensor("wo", [Dh, D], F32R, kind="ExternalInput")
    iden = nc.dram_tensor("iden", [Dh, Dh], F32R, kind="ExternalInput")
    y = nc.dram_tensor("y", [T, D], F32, kind="ExternalOutput")

    NSUB = TT // 512  # psum sub-tiles per projection token tile
    NQT = S // QTILE  # q tiles per batch
    NKC = S // KC  # k chunks per batch
    NB_ = S // 128  # 128-token tiles per batch

    with TileContext(nc) as tc:
        with (
            tc.tile_pool(name="const", bufs=1) as cpool,
            tc.tile_pool(name="persist", bufs=1) as ppool,
            tc.tile_pool(name="xin", bufs=2) as xpool,
            tc.tile_pool(name="qin", bufs=2) as qpool,
            tc.tile_pool(name="exps", bufs=4) as epool,
            tc.tile_pool(name="yout", bufs=2) as ypool,
            tc.tile_pool(name="dscr", bufs=1, space="DRAM") as dpool,
            tc.tile_pool(name="pa", bufs=2, space="PSUM") as pa,
            tc.tile_pool(name="lp", bufs=2, space="PSUM") as lp,
            tc.tile_pool(name="op", bufs=1, space="PSUM") as op,
        ):
            # ---- constants ----
            wkv_sb = cpool.tile([128, 4 * 2 * Dh], F32R)
            wq_sb = cpool.tile([128, 4 * Dh], F32R)
            wo_sb = cpool.tile([Dh, D], F32R)
            bkv_sb = cpool.tile([128, 1], F32)
            bq_sb = cpool.tile([Dh, 1], F32)
            ident_hi = cpool.tile([128, Dh], F32R)  # identity at partitions 64:128
            for c in range(4):
                nc.gpsimd.dma_start(wkv_sb[:, c * 128 : (c + 1) * 128], wkv[c])
                nc.gpsimd.dma_start(wq_sb[:, c * Dh : (c + 1) * Dh], wq[c])
            nc.gpsimd.dma_start(wo_sb[:], wo[:])
            nc.gpsimd.dma_start(bkv_sb[:], bkv[:])
            nc.gpsimd.dma_start(bq_sb[:], bq[:])
            nc.gpsimd.dma_start(ident_hi[64:128, :], iden[:])

            # ---- persistent intermediates ----
            kvt = ppool.tile([128, T], F32R)  # rows 0:64 K^T, rows 64:128 V^T
            qt = ppool.tile([Dh, T], F32R)
            vp = ppool.tile([128, VW * NKT], F32R)  # V' tiles [128, 65]
            outt = ppool.tile([VW, T], F32R)
            sums_sb = ppool.tile([128, NKT], F32R)
            recip = ppool.tile([128, NKT], F32)
            sums_dram = dpool.tile([1, T], F32R)

            for _ in range(reps):
                _lctx = tc.For_i(0, loop_n, 1) if loop_n else None
                if _lctx is not None:
                    _lctx.__enter__()
                nc.vector.memset(vp[:].bitcast(mybir.dt.uint32), 0x3F800000)

                def emit_proj(b):
                    base = b * S
                    for tt in range(S // TT):
                        t0 = base + tt * TT
                        xt_t = xpool.tile([128, 4 * TT], F32R, tag="xt")
                        qt_t = qpool.tile([128, 4 * TT], F32R, tag="qt")
                        nc.sync.dma_start(
                            xt_t[:],
                            xT[:, :, t0 : t0 + TT].rearrange("c p j -> p c j"),
                        )
                        nc.sync.dma_start(
                            qt_t[:],
                            qT[:, :, t0 : t0 + TT].rearrange("c p j -> p c j"),
                        )
                        for sub in range(NSUB):
                            s0 = t0 + sub * 512
                            o0 = sub * 512
                            kvp = pa.tile([128, 512], F32, tag="pa")
                            for c in range(4):
                                nc.tensor.matmul(
                                    kvp[:],
                                    wkv_sb[:, c * 128 : (c + 1) * 128],
                                    xt_t[:, c * TT + o0 : c * TT + o0 + 512],
                                    start=(c == 0),
                                    stop=(c == 3),
                                )
                            nc.vector.tensor_scalar_add(
                                kvt[:, s0 : s0 + 512], kvp[:], bkv_sb[:, 0:1]
                            )
                            qp = pa.tile([Dh, 512], F32, tag="pa")
                            for c in range(4):
                                nc.tensor.matmul(
                                    qp[:],
                                    wq_sb[:, c * Dh : (c + 1) * Dh],
                                    qt_t[:, c * TT + o0 : c * TT + o0 + 512],
                                    start=(c == 0),
                                    stop=(c == 3),
                                )
                            nc.vector.tensor_scalar_add(
                                qt[:, s0 : s0 + 512], qp[:], bq_sb[:, 0:1]
                            )
                            for kt in range(4):  # V' tiles for these 512 toks
                                kg = s0 // 128 + kt
                                k0 = s0 + kt * 128
                                vtp = pa.tile([128, Dh], F32R, tag="pa")
                                nc.tensor.transpose(
                                    vtp[:],
                                    kvt[64:128, k0 : k0 + 128],
                                    ident_hi[64:128, :],
                                )
                                nc.vector.tensor_copy(
                                    vp[:, kg * VW : kg * VW + Dh], vtp[:]
                                )

                def emit_attention_qt(b, qtile):
                    base = b * S
                    q0 = base + qtile * QTILE
                    po = op.tile([VW, QTILE], F32, tag="ot")
                    for kc in range(NKC):
                        kg = b * NB_ + kc
                        k0 = base + kc * KC
                        pl = lp.tile([128, QTILE], F32, tag="lt")
                        for hf in range(QTILE // 512):
                            nc.tensor.matmul(
                                pl[:, hf * 512 : (hf + 1) * 512],
                                kvt[0:64, k0 : k0 + KC],
                                qt[:, q0 + hf * 512 : q0 + (hf + 1) * 512],
                                start=True,
                                stop=True,
                            )
                        et = epool.tile([128, QTILE], F32R, tag="et")
                        nc.scalar.activation(
                            et[:], pl[:], mybir.ActivationFunctionType.Exp
                        )
                        for hf in range(QTILE // 512):
                            nc.tensor.matmul(
                                po[:, hf * 512 : (hf + 1) * 512],
                                vp[:, kg * VW : (kg + 1) * VW],
                                et[:, hf * 512 : (hf + 1) * 512],
                                start=(kc == 0),
                                stop=(kc == NKC - 1),
                            )
                    nc.vector.tensor_copy(outt[:, q0 : q0 + QTILE], po[:])

                def emit_sums_qt(b, qtile):
                    # softmax denominators for one q tile
                    base = b * S
                    q0 = base + qtile * QTILE
                    nqb = QTILE // 128
                    ft0 = q0 // 128
                    nc.sync.dma_start(
                        sums_dram[0:1, q0 : q0 + QTILE],
                        outt[Dh : Dh + 1, q0 : q0 + QTILE],
                    )
                    nc.sync.dma_start(
                        sums_sb[:, ft0 : ft0 + nqb],
                        sums_dram[0:1, q0 : q0 + QTILE].rearrange(
                            "o (f p) -> (o p) f", p=128
                        ),
                    )
                    nc.vector.reciprocal(
                        recip[:, ft0 : ft0 + nqb], sums_sb[:, ft0 : ft0 + nqb]
                    )

                def emit_y_qt(b, qtile, last=False):
                    # output projection for one q tile (2 groups of 512 tokens)
                    base = b * S
                    q0 = base + qtile * QTILE
                    nqb = QTILE // 128
                    ft0 = q0 // 128
                    for fg in range(nqb // 4):
                        g0 = q0 + fg * 512
                        yt = ypool.tile([128, 4 * 512], F32, tag="yt")
                        for j in range(4):
                            ft = ft0 + fg * 4 + j
                            f0 = ft * 128
                            py = pa.tile([128, 512], F32, tag="pa")
                            nc.tensor.matmul(
                                py[:],
                                outt[0:Dh, f0 : f0 + 128],
                                wo_sb[:],
                                start=True,
                                stop=True,
                            )
                            if last and j % 2 == 0:
                                nc.scalar.activation(
                                    yt[:, j * 512 : (j + 1) * 512],
                                    py[:],
                                    mybir.ActivationFunctionType.Copy,
                                    scale=recip[:, ft : ft + 1],
                                )
                            else:
                                nc.vector.tensor_scalar_mul(
                                    yt[:, j * 512 : (j + 1) * 512],
                                    py[:],
                                    recip[:, ft : ft + 1],
                                )
                        nc.sync.dma_start(
                            y[g0 : g0 + 512, :].rearrange("(j p) c -> p j c", p=128),
                            yt[:],
                        )

                # software-pipelined emission: attention(b) || y(b-1) || proj(b+1)
                emit_proj(0)
                for b in range(B):
                    emit_attention_qt(b, 0)
                    emit_sums_qt(b, 0)
                    if b + 1 < B:
                        emit_proj(b + 1)
                    emit_attention_qt(b, 1)
                    emit_sums_qt(b, 1)
                    if b > 0:
                        for qtile in range(NQT):
                            emit_y_qt(b - 1, qtile)
                for qtile in range(NQT):
                    emit_y_qt(B - 1, qtile, last=(qtile == NQT - 1))
                if _lctx is not None:
                    _lctx.__exit__(None, None, None)

    _split_multi_waits(nc)
    return nc


_CACHE: dict = {}


def _prep_inputs(x, q, Wq, bq, Wk, bk, Wv, bv, Wo, bo):
    x = np.asarray(x, np.float32)
    q = np.asarray(q, np.float32)
    Wq, bq = np.asarray(Wq, np.float32), np.asarray(bq, np.float32)
    Wk, bk = np.asarray(Wk, np.float32), np.asarray(bk, np.float32)
    Wv, bv = np.asarray(Wv, np.float32), np.asarray(bv, np.float32)
    Wo = np.asarray(Wo, np.float32)

    scale = 1.0 / np.sqrt(np.float32(Dh))
    xT4 = np.ascontiguousarray(x.reshape(T, D).T.reshape(4, 128, T))
    qT4 = np.ascontiguousarray(q.reshape(T, D).T.reshape(4, 128, T))
    in_maps = []
    for h in range(NCORES):
        sl = slice(h * Dh, (h + 1) * Dh)
        wkv_h = np.concatenate([Wk[sl].T, Wv[sl].T], axis=1)  # [512, 128]
        bkv_h = np.concatenate([bk[sl], bv[sl]])[:, None]  # [128, 1]
        wq_h = (Wq[sl] * scale).T  # [512, 64]
        bq_h = (bq[sl] * scale)[:, None]
        wo_h = np.ascontiguousarray(Wo[:, sl].T)  # [64, 512]
        in_maps.append(
            {
                "xT": xT4,
                "qT": qT4,
                "wkv": np.ascontiguousarray(wkv_h.reshape(4, 128, 128), np.float32),
                "bkv": np.ascontiguousarray(bkv_h, dtype=np.float32),
                "wq": np.ascontiguousarray(wq_h.reshape(4, 128, Dh), np.float32),
                "bq": np.ascontiguousarray(bq_h, dtype=np.float32),
                "wo": wo_h,
                "iden": np.eye(Dh, dtype=np.float32),
            }
        )
    return in_maps


def kernel(x, q, Wq, bq, Wk, bk, Wv, bv, Wo, bo):
    _install_drain_patch()
    if "nc" not in _CACHE:
        _CACHE["nc"] = _build()
    nc = _CACHE["nc"]
    in_maps = _prep_inputs(x, q, Wq, bq, Wk, bk, Wv, bv, Wo, bo)
    res = run_bass_kernel_spmd(nc, in_maps, core_ids=list(range(NCORES)))
    y = np.zeros((T, D), np.float64)
    for r in res.results:
        y += r["y"].astype(np.float64)
    y = (y + np.asarray(bo, np.float32).astype(np.float64)).astype(np.float32)
    return y.reshape(B, S, D)



# revision 3
# speedup vs baseline: 1.0492x; 1.0492x over previous
"""Trainium2 Bass kernel for 8-head MHA (B=4, S=2048, D=512), v2.

Sharding: 2D over (batch, head-group). Core c owns batch b=c//2 and heads
4g..4g+3 where g=c%2. Per-core inputs are x[b], q[b] (4MB each) and the
weight slices for its 4 heads -- ~13MB DMA/core vs 48MB for pure
head-parallel. Host sums the 2 partial y's per batch and adds bo.

Per core (all matmuls fp32r, 1 cycle/row at moving>=256):
  K^T pair-stacked: kt[128, S] per pair = [K^T_h0; K^T_h1] (full PE util)
  Q^T pair-stacked likewise (halves Q-proj rows vs 1-head-per-core)
  V computed DIRECTLY in [token, dh] orientation: stationary x^T chunk,
    moving Wv^T[128,256] (all 4 heads) -> no PE transpose needed
  logits^T[k,q] per head via PE row-tile at partition 64*(h%2)
  expT on ACT ([128,1024] tiles); attnV with V' ones-column accumulating
    sumexp in row 64; normalization = reciprocal + DMA partition-broadcast
    of the recip row + in-place tensor_tensor multiply
  y pair-accumulated: py[128t,512] += outp_pair[128,t128].T @ Wo_pair[128,512]
    (2 matmuls/tile instead of 4)
"""

import numpy as np

import concourse.bass as bass
import concourse.mybir as mybir
from concourse.tile import TileContext
from concourse.bass_utils import run_bass_kernel_spmd

# ---------------------------------------------------------------------------
# Workaround: this container's walrus rejects >1 sync wait on an InstDrain
# (TPB_CTRL). Split the TileContext exit-drain waits across single-wait NOPs.
_PATCHED = False


def _install_drain_patch():
    global _PATCHED
    if _PATCHED:
        return
    from concourse.vector_clock import ScopedClock, VectorClock

    def _split_drain_and_barrier(self, tick_clock, wait_clock):
        g = tick_clock.global_clock
        n = len(g)
        for i in range(n):
            t = g[i]
            if t > 0:
                vec = [0] * n
                vec[i] = t
                nop = self.nc.sync.nop(nofuse=True, hint=f"drain_wait_p{i}")
                wait_clock.add_sem_waits(
                    nop.ins, ScopedClock({None: VectorClock(vec)})
                )
        self.nc.sync.drain()
        self.nc.all_engine_barrier()
        assert self.sems is not None
        popped = self.nc._tile_sem_poison_stack.pop()
        assert popped is self._sem_poison
        self.nc.clear_and_free_semaphores(list(self.sems.allocated().values()))
        self.nc.all_engine_barrier()

    TileContext._drain_and_barrier = _split_drain_and_barrier
    _PATCHED = True


def _split_multi_waits(nc):
    """This walrus accepts at most ONE sync wait per instruction. Hoist extra
    waits onto same-engine NOPs inserted immediately before the instruction
    (same-engine program order preserves semantics)."""
    n_split = 0
    for blk in nc.m.functions[0].blocks:
        il = blk.instructions
        i = 0
        while i < len(il):
            inst = il[i]
            try:
                si = inst.sync_info
            except AttributeError:
                si = None
            if si is not None and si.on_wait is not None and len(si.on_wait) > 1:
                waits = list(si.on_wait)
                for j, w in enumerate(waits[:-1]):
                    nop = mybir.InstNoOp(
                        name=f"{inst.name}_hw{j}",
                        sync_info=mybir.SyncInfo(on_wait=[w], on_update=[]),
                        bass_nofuse=True,
                        engine=inst.engine,
                    )
                    il.insert(i, nop)
                    i += 1
                inst.sync_info = mybir.SyncInfo(
                    on_wait=[waits[-1]], on_update=list(si.on_update)
                )
                n_split += 1
            i += 1
    return n_split


# ---------------------------------------------------------------------------
B, S, D, H = 4, 2048, 512, 8
Dh = D // H  # 64
NCORES = 8
HPC = 4  # heads per core

F32 = mybir.dt.float32
F32R = mybir.dt.float32r
BF16 = mybir.dt.bfloat16
NP_BF16 = mybir.dt.np(BF16)

TT = 512  # projection token tile
NTT = S // TT  # 4
QTILE = 1024
NQT = S // QTILE  # 2
KC = 128  # keys per chunk
NKC = S // KC  # 16
VW = Dh + 1  # V' width per k-group (ones column at 64)
VG = HPC * VW  # 260: per k-group, 4 heads' V' blocks

# Schraudolph exp on DVE in bf16 bit space (offloads ACT, the attention-phase
# pacer): bf16_bits(exp(x)) ~= int16(x*128/ln2 + (127*128 - c)); c tuned
# numerically for min softmax-output error (see session notes, c=7.5)
SCHR_KC = (5, 10, 13)  # which of the 16 k-chunks per block go to DVE
SCHR_A = float(128.0 / np.log(2.0))
SCHR_B = 127.0 * 128.0 - 7.5


def _build() -> bass.Bass:
    nc = bass.Bass(name="mha2")
    xT = nc.dram_tensor("xT", [4, 128, S], BF16, kind="ExternalInput")
    qT = nc.dram_tensor("qT", [4, 128, S], BF16, kind="ExternalInput")
    wk = nc.dram_tensor("wk", [8, 128, 128], BF16, kind="ExternalInput")
    wq = nc.dram_tensor("wq", [8, 128, 128], BF16, kind="ExternalInput")
    wvT = nc.dram_tensor("wvT", [4, 128, 256], BF16, kind="ExternalInput")
    wo = nc.dram_tensor("wo", [2, 128, 512], F32R, kind="ExternalInput")
    bk = nc.dram_tensor("bk", [128, 2], F32, kind="ExternalInput")
    bq = nc.dram_tensor("bq", [128, 2], F32, kind="ExternalInput")
    bv = nc.dram_tensor("bv", [128, 512], F32, kind="ExternalInput")
    y = nc.dram_tensor("y", [S, D], BF16, kind="ExternalOutput")

    with TileContext(nc) as tc:
        with (
            tc.tile_pool(name="const", bufs=1) as cpool,
            tc.tile_pool(name="persist", bufs=1) as ppool,
            tc.tile_pool(name="xin", bufs=4) as xpool,
            tc.tile_pool(name="qin", bufs=4) as qpool,
            tc.tile_pool(name="exps", bufs=6) as epool,
            tc.tile_pool(name="stgp", bufs=2) as spool,
            tc.tile_pool(name="srow", bufs=2) as srpool,
            tc.tile_pool(name="rrow", bufs=2) as rrpool,
            tc.tile_pool(name="yout", bufs=4) as ypool,
            tc.tile_pool(name="pa", bufs=2, space="PSUM") as pa,
            tc.tile_pool(name="lp", bufs=2, space="PSUM") as lp,
            tc.tile_pool(name="op", bufs=1, space="PSUM") as op,
        ):
            # ---- constants ----
            wk_sb = cpool.tile([128, 8 * 128], BF16)  # [pair*4+chunk] blocks
            wq_sb = cpool.tile([128, 8 * 128], BF16)
            wvT_sb = cpool.tile([128, 4 * 256], BF16)  # per chunk
            wo_sb = cpool.tile([128, 2 * 512], F32R)  # per pair
            bk_sb = cpool.tile([128, 2], F32)
            bq_sb = cpool.tile([128, 2], F32)
            bv_sb = cpool.tile([128, 512], F32)  # doubled for j-pair evac
            ones_sb = cpool.tile([128, 64], F32R)
            nc.vector.memset(ones_sb[:].bitcast(mybir.dt.uint32), 0x3F800000)
            # fast SP-queue combined weight loads, critical-path order
            # (x0/q0 tile DMAs are emitted between these via emit_load)
            # small biases off the critical path on the SWDGE queue
            nc.gpsimd.dma_start(bk_sb[:], bk[:])
            nc.gpsimd.dma_start(bq_sb[:], bq[:])
            nc.gpsimd.dma_start(bv_sb[:], bv[:])

            def emit_early_weights():
                nc.sync.dma_start(wvT_sb[:], wvT.rearrange("c p j -> p c j"))
                nc.sync.dma_start(wk_sb[:], wk.rearrange("i p j -> p i j"))

            def emit_late_weights():
                nc.sync.dma_start(wq_sb[:], wq.rearrange("i p j -> p i j"))
                nc.sync.dma_start(wo_sb[:], wo.rearrange("i p j -> p i j"))

            # ---- persistent intermediates ----
            kt = ppool.tile([128, 2 * S], F32R)  # pair-major K^T
            qt = ppool.tile([128, 2 * S], F32R)  # pair-major Q^T
            vp = ppool.tile([128, NKC * VG], BF16)  # kgroup-major V' blocks
            outp = ppool.tile([128, 2 * S], F32R)  # pair-major normalized out^T

            nc.vector.memset(vp[:].bitcast(mybir.dt.uint16), 0x3F80)

            xts: list = []
            qis: list = []

            def emit_load_x(tti):
                t0 = tti * TT
                xt = xpool.tile([128, 4 * TT], BF16, tag="xt")
                nc.sync.dma_start(
                    xt[:], xT[:, :, t0 : t0 + TT].rearrange("c p j -> p c j")
                )
                xts.append(xt)

            def emit_load_q(tti):
                t0 = tti * TT
                qi = qpool.tile([128, 4 * TT], BF16, tag="qi")
                nc.sync.dma_start(
                    qi[:], qT[:, :, t0 : t0 + TT].rearrange("c p j -> p c j")
                )
                qis.append(qi)

            def emit_v(tti):
                t0 = tti * TT
                xt = xts[tti]
                # V direct: psum[128 toks, 2*256] = x[toks,:] @ Wv_g^T for a
                # j-pair of 128-token groups packed in one psum bank
                for jp in range(2):
                    kg0 = tti * 4 + jp * 2
                    pv = pa.tile([128, 512], F32, tag="pa")
                    for j in range(2):
                        for c in range(4):
                            nc.tensor.matmul(
                                pv[:, j * 256 : j * 256 + 256],
                                xt[
                                    :,
                                    c * TT
                                    + (jp * 2 + j) * 128 : c * TT
                                    + (jp * 2 + j + 1) * 128,
                                ],
                                wvT_sb[:, c * 256 : (c + 1) * 256],
                                start=(c == 0),
                                stop=(c == 3),
                            )
                    nc.vector.tensor_tensor(
                        vp[:, kg0 * VG : (kg0 + 2) * VG].rearrange(
                            "p (j h m) -> p j h m", j=2, h=4
                        )[:, :, :, 0:Dh],
                        pv[:].rearrange("p (j h m) -> p j h m", j=2, h=4),
                        bv_sb[:].rearrange("p (j h m) -> p j h m", j=2, h=4),
                        op=mybir.AluOpType.add,
                    )
            def emit_k(tti, p):
                t0 = tti * TT
                pk = pa.tile([128, 512], F32, tag="pa")
                for c in range(4):
                    nc.tensor.matmul(
                        pk[:],
                        wk_sb[:, (p * 4 + c) * 128 : (p * 4 + c + 1) * 128],
                        xts[tti][:, c * TT : (c + 1) * TT],
                        start=(c == 0),
                        stop=(c == 3),
                    )
                nc.vector.tensor_scalar_add(
                    kt[:, p * S + t0 : p * S + t0 + TT], pk[:], bk_sb[:, p : p + 1]
                )

            def emit_q(tti, p):
                t0 = tti * TT
                pq = pa.tile([128, 512], F32, tag="pa")
                for c in range(4):
                    nc.tensor.matmul(
                        pq[:],
                        wq_sb[:, (p * 4 + c) * 128 : (p * 4 + c + 1) * 128],
                        qis[tti][:, c * TT : (c + 1) * TT],
                        start=(c == 0),
                        stop=(c == 3),
                    )
                nc.vector.tensor_scalar_add(
                    qt[:, p * S + t0 : p * S + t0 + TT], pq[:], bq_sb[:, p : p + 1]
                )

            def emit_y(qtile, ft, wide=False, act_copy=False):
                # output projection for one 128-token tile; `wide` borrows a
                # half-used logits-shaped psum so the tail pipelines deeper
                f0 = qtile * QTILE + ft * 128
                if wide:
                    pyw = lp.tile([128, QTILE], F32, tag="pl")
                    py = pyw[:, 0:512]
                else:
                    py = pa.tile([128, 512], F32, tag="pa")
                for p in range(2):
                    nc.tensor.matmul(
                        py[:],
                        outp[:, p * S + f0 : p * S + f0 + 128],
                        wo_sb[:, p * 512 : (p + 1) * 512],
                        start=(p == 0),
                        stop=(p == 1),
                    )
                yt = ypool.tile([128, 512], BF16, tag="yt")
                if act_copy:
                    nc.scalar.copy(yt[:], py[:])
                else:
                    nc.vector.tensor_copy(yt[:], py[:])
                nc.sync.dma_start(y[f0 : f0 + 128, :], yt[:])

            pending = []

            def emit_logits(h, qtile, kc):
                hh = h % 2
                p, base = h // 2, 64 * hh
                q0 = qtile * QTILE
                et = epool.tile([128, QTILE], BF16, tag="et")
                pl = lp.tile([128, QTILE], F32, tag="pl")
                for hf in range(2):
                    nc.tensor.matmul(
                        pl[:, hf * 512 : (hf + 1) * 512],
                        kt[base : base + 64, p * S + kc * KC : p * S + (kc + 1) * KC],
                        qt[
                            base : base + 64,
                            p * S + q0 + hf * 512 : p * S + q0 + (hf + 1) * 512,
                        ],
                        start=True,
                        stop=True,
                    )
                nc.scalar.activation(
                    et[:], pl[:], mybir.ActivationFunctionType.Exp
                )
                return et

            def emit_block_end(h, qtile, po):
                # evacuate po -> SBUF staging (frees the psum bank fast);
                # sums row to partition 0 by DMA, then reciprocal
                p, hh = h // 2, h % 2
                ocol = p * S + qtile * QTILE
                stg = spool.tile([VW, QTILE], F32, tag="stg")
                nc.vector.tensor_copy(stg[:], po[:])
                rrow = rrpool.tile([VW, QTILE], F32R, tag="rrow")
                with nc.allow_low_precision(reason="f32r recip row for PE"):
                    nc.vector.reciprocal(rrow[64:65, :], stg[64:65, :])

                def finish(ocol=ocol, rrow=rrow, stg=stg, hh=hh):
                    # broadcast recip row across 64 partitions via a
                    # [1,64]-ones stationary matmul (standard (0,0) tile),
                    # then scale; odd heads reach partitions 64:128 via DMA
                    for hf in range(2):
                        rb = pa.tile([128, 512], F32, tag="pa")
                        nc.tensor.matmul(
                            rb[0:64, :],
                            ones_sb[64:65, :],
                            rrow[64:65, hf * 512 : (hf + 1) * 512],
                            start=True,
                            stop=True,
                        )
                        ssl = stg[0:64, hf * 512 : (hf + 1) * 512]
                        if hh == 0:
                            nc.vector.tensor_tensor(
                                outp[
                                    0:64,
                                    ocol + hf * 512 : ocol + (hf + 1) * 512,
                                ],
                                ssl,
                                rb[0:64, :],
                                op=mybir.AluOpType.mult,
                            )
                        else:
                            nc.vector.tensor_tensor(
                                ssl, ssl, rb[0:64, :], op=mybir.AluOpType.mult
                            )
                    if hh == 1:
                        nc.sync.dma_start(
                            outp[64:128, ocol : ocol + QTILE].bitcast(F32),
                            stg[0:64, :],
                        )

                pending.append(finish)

            def emit_attention_stream():
                # one continuous software pipeline over all (qt, h, kc)
                # steps: logits/exp of step i+1 are emitted before attnV of
                # step i, ACROSS block boundaries, so the ACT exp stream
                # never drains while a block finishes. qt1 ends on an even
                # head so the final outp write needs no partition-move DMA.
                steps = [
                    (h, qtile, kc)
                    for qtile in range(NQT)
                    for h in ((0, 1, 2, 3) if qtile == 0 else (0, 1, 3, 2))
                    for kc in range(NKC)
                ]
                po = None
                et_cur = emit_logits(*steps[0])
                for i, (h, qtile, kc) in enumerate(steps):
                    et_next = (
                        emit_logits(*steps[i + 1]) if i + 1 < len(steps) else None
                    )
                    # extra PE work goes before attnV so it never delays the
                    # next logits pair that paces the ACT exp stream
                    if kc == 2 and pending:
                        pending.pop(0)()
                    if kc in (3, 7, 11, 14) and spill:
                        spill.pop(0)()
                    if qtile == 1 and kc == 5:
                        emit_y(0, h * 2)
                    if qtile == 1 and kc == 11:
                        emit_y(0, h * 2 + 1)
                    if kc == 0:
                        po = op.tile([VW, QTILE], F32, tag="po")
                    for hf in range(2):
                        nc.tensor.matmul(
                            po[:, hf * 512 : (hf + 1) * 512],
                            vp[:, kc * VG + h * VW : kc * VG + (h + 1) * VW],
                            et_cur[:, hf * 512 : (hf + 1) * 512],
                            start=(kc == 0),
                            stop=(kc == NKC - 1),
                        )
                    et_cur = et_next
                    if kc == NKC - 1:
                        emit_block_end(h, qtile, po)

            # phase 1: everything attention blocks h0/h1 (pair 0) need,
            # DMAs ordered so the first V/K/Q matmuls unblock earliest
            emit_load_x(0)
            emit_early_weights()
            emit_load_q(0)
            nc.sync.dma_start(wq_sb[:], wq.rearrange("i p j -> p i j"))
            emit_load_x(1)
            emit_load_q(1)
            nc.sync.dma_start(wo_sb[:], wo.rearrange("i p j -> p i j"))
            emit_load_x(2)
            emit_load_x(3)
            emit_load_q(2)
            emit_load_q(3)
            for tti in range(NTT):
                emit_v(tti)
                if tti < 2:
                    emit_k(tti, 0)
                    emit_q(tti, 0)
            # phase 2: spilled into the early attention blocks' PE slack
            spill = [
                lambda: emit_k(2, 0),
                lambda: emit_k(3, 0),
                lambda: emit_q(0, 1),
                lambda: emit_k(0, 1),
                lambda: emit_k(1, 1),
                lambda: emit_q(1, 1),
                lambda: emit_k(2, 1),
                lambda: emit_k(3, 1),
                lambda: emit_q(2, 0),
                lambda: emit_q(3, 0),
                lambda: emit_q(2, 1),
                lambda: emit_q(3, 1),
            ]
            emit_attention_stream()
            assert not spill
            while pending:
                pending.pop(0)()
            # tail: alternate y psums between the pa ring and the now-idle
            # logits ring so the eight final tiles pipeline 4-deep; evac
            # copies ride the now-idle ACT engine
            for ft in range(8):
                emit_y(NQT - 1, ft, wide=(ft % 2 == 1), act_copy=True)

    _split_multi_waits(nc)
    return nc


_CACHE: dict = {}


def _prep_inputs(x, q, Wq, bq, Wk, bk, Wv, bv, Wo, bo):
    x = np.asarray(x, np.float32)
    q = np.asarray(q, np.float32)
    Wq, bq = np.asarray(Wq, np.float32), np.asarray(bq, np.float32)
    Wk, bk = np.asarray(Wk, np.float32), np.asarray(bk, np.float32)
    Wv, bv = np.asarray(Wv, np.float32), np.asarray(bv, np.float32)
    Wo = np.asarray(Wo, np.float32)

    scale = np.float32(1.0 / np.sqrt(np.float32(Dh)))
    Wqs, bqs = Wq * scale, bq * scale

    xTb = [
        np.ascontiguousarray(x[b].T.reshape(4, 128, S)).astype(NP_BF16)
        for b in range(B)
    ]
    qTb = [
        np.ascontiguousarray(q[b].T.reshape(4, 128, S)).astype(NP_BF16)
        for b in range(B)
    ]

    in_maps = []
    for c in range(NCORES):
        b, g = c // 2, c % 2
        hs = slice(g * 256, (g + 1) * 256)  # 4 heads' dims
        Wk_g, Wq_g, Wv_g = Wk[hs], Wqs[hs], Wv[hs]  # [256, 512]
        bk_g, bq_g, bv_g = bk[hs], bqs[hs], bv[hs]  # [256]
        # [pair, chunk, 128, 128]: chunk c of (pair rows).T
        wk_a = np.stack(
            [
                Wk_g[p * 128 : (p + 1) * 128].T.reshape(4, 128, 128)[ch]
                for p in range(2)
                for ch in range(4)
            ]
        )
        wq_a = np.stack(
            [
                Wq_g[p * 128 : (p + 1) * 128].T.reshape(4, 128, 128)[ch]
                for p in range(2)
                for ch in range(4)
            ]
        )
        wvT_a = np.ascontiguousarray(Wv_g.T.reshape(4, 128, 256))
        wo_a = np.stack(
            [
                np.ascontiguousarray(Wo[:, hs][:, p * 128 : (p + 1) * 128].T)
                for p in range(2)
            ]
        )
        bk_a = np.ascontiguousarray(bk_g.reshape(2, 128).T)
        bq_a = np.ascontiguousarray(bq_g.reshape(2, 128).T)
        bv_a = np.ascontiguousarray(np.tile(bv_g[None, :], (128, 2)))
        in_maps.append(
            {
                "xT": xTb[b],
                "qT": qTb[b],
                "wk": np.ascontiguousarray(wk_a).astype(NP_BF16),
                "wq": np.ascontiguousarray(wq_a).astype(NP_BF16),
                "wvT": np.ascontiguousarray(wvT_a).astype(NP_BF16),
                "wo": np.ascontiguousarray(wo_a, np.float32),
                "bk": np.ascontiguousarray(bk_a, np.float32),
                "bq": np.ascontiguousarray(bq_a, np.float32),
                "bv": np.ascontiguousarray(bv_a, np.float32),
            }
        )
    return in_maps


def kernel(x, q, Wq, bq, Wk, bk, Wv, bv, Wo, bo):
    _install_drain_patch()
    if "nc" not in _CACHE:
        _CACHE["nc"] = _build()
    nc = _CACHE["nc"]
    in_maps = _prep_inputs(x, q, Wq, bq, Wk, bk, Wv, bv, Wo, bo)
    res = run_bass_kernel_spmd(nc, in_maps, core_ids=list(range(NCORES)))
    bo64 = np.asarray(bo, np.float32).astype(np.float64)
    out = np.empty((B, S, D), np.float32)
    for b in range(B):
        acc = (
            res.results[2 * b]["y"].astype(np.float64)
            + res.results[2 * b + 1]["y"].astype(np.float64)
            + bo64
        )
        out[b] = acc.astype(np.float32)
    return out


# revision 4
# speedup vs baseline: 1.0784x; 1.0279x over previous
"""Trainium2 Bass kernel for 8-head MHA (B=4, S=2048, D=512), v2.

Sharding: 2D over (batch, head-group). Core c owns batch b=c//2 and heads
4g..4g+3 where g=c%2. Per-core inputs are x[b], q[b] (4MB each) and the
weight slices for its 4 heads -- ~13MB DMA/core vs 48MB for pure
head-parallel. Host sums the 2 partial y's per batch and adds bo.

Per core (all matmuls fp32r, 1 cycle/row at moving>=256):
  K^T pair-stacked: kt[128, S] per pair = [K^T_h0; K^T_h1] (full PE util)
  Q^T pair-stacked likewise (halves Q-proj rows vs 1-head-per-core)
  V computed DIRECTLY in [token, dh] orientation: stationary x^T chunk,
    moving Wv^T[128,256] (all 4 heads) -> no PE transpose needed
  logits^T[k,q] per head via PE row-tile at partition 64*(h%2)
  expT on ACT ([128,1024] tiles); attnV with V' ones-column accumulating
    sumexp in row 64; normalization = reciprocal + DMA partition-broadcast
    of the recip row + in-place tensor_tensor multiply
  y pair-accumulated: py[128t,512] += outp_pair[128,t128].T @ Wo_pair[128,512]
    (2 matmuls/tile instead of 4)
"""

import numpy as np

import concourse.bass as bass
import concourse.mybir as mybir
from concourse.tile import TileContext
from concourse.bass_utils import run_bass_kernel_spmd

# ---------------------------------------------------------------------------
# Workaround: this container's walrus rejects >1 sync wait on an InstDrain
# (TPB_CTRL). Split the TileContext exit-drain waits across single-wait NOPs.
_PATCHED = False


def _install_drain_patch():
    global _PATCHED
    if _PATCHED:
        return
    from concourse.vector_clock import ScopedClock, VectorClock

    def _split_drain_and_barrier(self, tick_clock, wait_clock):
        g = tick_clock.global_clock
        n = len(g)
        for i in range(n):
            t = g[i]
            if t > 0:
                vec = [0] * n
                vec[i] = t
                nop = self.nc.sync.nop(nofuse=True, hint=f"drain_wait_p{i}")
                wait_clock.add_sem_waits(
                    nop.ins, ScopedClock({None: VectorClock(vec)})
                )
        self.nc.sync.drain()
        self.nc.all_engine_barrier()
        assert self.sems is not None
        popped = self.nc._tile_sem_poison_stack.pop()
        assert popped is self._sem_poison
        self.nc.clear_and_free_semaphores(list(self.sems.allocated().values()))
        self.nc.all_engine_barrier()

    TileContext._drain_and_barrier = _split_drain_and_barrier
    _PATCHED = True


def _split_multi_waits(nc):
    """This walrus accepts at most ONE sync wait per instruction. Hoist extra
    waits onto same-engine NOPs inserted immediately before the instruction
    (same-engine program order preserves semantics)."""
    n_split = 0
    for blk in nc.m.functions[0].blocks:
        il = blk.instructions
        i = 0
        while i < len(il):
            inst = il[i]
            try:
                si = inst.sync_info
            except AttributeError:
                si = None
            if si is not None and si.on_wait is not None and len(si.on_wait) > 1:
                waits = list(si.on_wait)
                for j, w in enumerate(waits[:-1]):
                    nop = mybir.InstNoOp(
                        name=f"{inst.name}_hw{j}",
                        sync_info=mybir.SyncInfo(on_wait=[w], on_update=[]),
                        bass_nofuse=True,
                        engine=inst.engine,
                    )
                    il.insert(i, nop)
                    i += 1
                inst.sync_info = mybir.SyncInfo(
                    on_wait=[waits[-1]], on_update=list(si.on_update)
                )
                n_split += 1
            i += 1
    return n_split


# ---------------------------------------------------------------------------
B, S, D, H = 4, 2048, 512, 8
Dh = D // H  # 64
NCORES = 8
HPC = 4  # heads per core

F32 = mybir.dt.float32
F32R = mybir.dt.float32r
BF16 = mybir.dt.bfloat16
NP_BF16 = mybir.dt.np(BF16)

TT = 512  # projection token tile
NTT = S // TT  # 4
QTILE = 1024
NQT = S // QTILE  # 2
KC = 128  # keys per chunk
NKC = S // KC  # 16
VW = Dh + 1  # V' width per k-group (ones column at 64)
VG = HPC * VW  # 260: per k-group, 4 heads' V' blocks

# Schraudolph exp on DVE in bf16 bit space (offloads ACT, the attention-phase
# pacer): bf16_bits(exp(x)) ~= int16(x*128/ln2 + (127*128 - c)); c tuned
# numerically for min softmax-output error (see session notes, c=7.5)
SCHR_KC = (5, 10, 13)  # which of the 16 k-chunks per block go to DVE
SCHR_A = float(128.0 / np.log(2.0))
SCHR_B = 127.0 * 128.0 - 7.5


def _build() -> bass.Bass:
    nc = bass.Bass(name="mha2")
    xT = nc.dram_tensor("xT", [4, 128, S], BF16, kind="ExternalInput")
    qT = nc.dram_tensor("qT", [4, 128, S], BF16, kind="ExternalInput")
    wk = nc.dram_tensor("wk", [8, 128, 128], BF16, kind="ExternalInput")
    wq = nc.dram_tensor("wq", [8, 128, 128], BF16, kind="ExternalInput")
    wvT = nc.dram_tensor("wvT", [4, 128, 256], BF16, kind="ExternalInput")
    wo = nc.dram_tensor("wo", [2, 128, 512], F32R, kind="ExternalInput")
    bk = nc.dram_tensor("bk", [128, 2], F32, kind="ExternalInput")
    bq = nc.dram_tensor("bq", [128, 2], F32, kind="ExternalInput")
    bv = nc.dram_tensor("bv", [128, 512], F32, kind="ExternalInput")
    y = nc.dram_tensor("y", [S, D], BF16, kind="ExternalOutput")

    with TileContext(nc) as tc:
        with (
            tc.tile_pool(name="const", bufs=1) as cpool,
            tc.tile_pool(name="persist", bufs=1) as ppool,
            tc.tile_pool(name="xin", bufs=4) as xpool,
            tc.tile_pool(name="qin", bufs=4) as qpool,
            tc.tile_pool(name="exps", bufs=6) as epool,
            tc.tile_pool(name="stgp", bufs=2) as spool,
            tc.tile_pool(name="srow", bufs=2) as srpool,
            tc.tile_pool(name="rrow", bufs=2) as rrpool,
            tc.tile_pool(name="yout", bufs=4) as ypool,
            tc.tile_pool(name="pa", bufs=2, space="PSUM") as pa,
            tc.tile_pool(name="lp", bufs=2, space="PSUM") as lp,
            tc.tile_pool(name="op", bufs=1, space="PSUM") as op,
        ):
            # ---- constants ----
            wk_sb = cpool.tile([128, 8 * 128], BF16)  # [pair*4+chunk] blocks
            wq_sb = cpool.tile([128, 8 * 128], BF16)
            wvT_sb = cpool.tile([128, 4 * 256], BF16)  # per chunk
            wo_sb = cpool.tile([128, 2 * 512], F32R)  # per pair
            bk_sb = cpool.tile([128, 2], F32)
            bq_sb = cpool.tile([128, 2], F32)
            bv_sb = cpool.tile([128, 512], F32)  # doubled for j-pair evac
            ones_sb = cpool.tile([128, 64], F32R)
            nc.vector.memset(ones_sb[:].bitcast(mybir.dt.uint32), 0x3F800000)
            # fast SP-queue combined weight loads, critical-path order
            # (x0/q0 tile DMAs are emitted between these via emit_load)
            # small biases off the critical path on the SWDGE queue
            nc.gpsimd.dma_start(bk_sb[:], bk[:])
            nc.gpsimd.dma_start(bq_sb[:], bq[:])
            nc.gpsimd.dma_start(bv_sb[:], bv[:])

            def emit_early_weights():
                nc.sync.dma_start(wvT_sb[:], wvT.rearrange("c p j -> p c j"))
                nc.sync.dma_start(wk_sb[:], wk.rearrange("i p j -> p i j"))

            def emit_late_weights():
                nc.sync.dma_start(wq_sb[:], wq.rearrange("i p j -> p i j"))
                nc.sync.dma_start(wo_sb[:], wo.rearrange("i p j -> p i j"))

            # ---- persistent intermediates ----
            kt = ppool.tile([128, 2 * S], F32R)  # pair-major K^T
            qt = ppool.tile([128, 2 * S], F32R)  # pair-major Q^T
            vp = ppool.tile([128, NKC * VG], BF16)  # kgroup-major V' blocks
            outp = ppool.tile([128, 2 * S], F32R)  # pair-major normalized out^T

            nc.vector.memset(vp[:].bitcast(mybir.dt.uint16), 0x3F80)

            xts: list = []
            qis: list = []

            def emit_load_x(tti):
                t0 = tti * TT
                xt = xpool.tile([128, 4 * TT], BF16, tag="xt")
                nc.sync.dma_start(
                    xt[:], xT[:, :, t0 : t0 + TT].rearrange("c p j -> p c j")
                )
                xts.append(xt)

            def emit_load_q(tti):
                t0 = tti * TT
                qi = qpool.tile([128, 4 * TT], BF16, tag="qi")
                nc.sync.dma_start(
                    qi[:], qT[:, :, t0 : t0 + TT].rearrange("c p j -> p c j")
                )
                qis.append(qi)

            def emit_v(tti):
                t0 = tti * TT
                xt = xts[tti]
                # V direct: psum[128 toks, 2*256] = x[toks,:] @ Wv_g^T for a
                # j-pair of 128-token groups packed in one psum bank
                for jp in range(2):
                    kg0 = tti * 4 + jp * 2
                    pv = pa.tile([128, 512], F32, tag="pa")
                    for j in range(2):
                        for c in range(4):
                            nc.tensor.matmul(
                                pv[:, j * 256 : j * 256 + 256],
                                xt[
                                    :,
                                    c * TT
                                    + (jp * 2 + j) * 128 : c * TT
                                    + (jp * 2 + j + 1) * 128,
                                ],
                                wvT_sb[:, c * 256 : (c + 1) * 256],
                                start=(c == 0),
                                stop=(c == 3),
                            )
                    nc.vector.tensor_tensor(
                        vp[:, kg0 * VG : (kg0 + 2) * VG].rearrange(
                            "p (j h m) -> p j h m", j=2, h=4
                        )[:, :, :, 0:Dh],
                        pv[:].rearrange("p (j h m) -> p j h m", j=2, h=4),
                        bv_sb[:].rearrange("p (j h m) -> p j h m", j=2, h=4),
                        op=mybir.AluOpType.add,
                    )
            def emit_k(tti, p):
                t0 = tti * TT
                pk = pa.tile([128, 512], F32, tag="pa")
                for c in range(4):
                    nc.tensor.matmul(
                        pk[:],
                        wk_sb[:, (p * 4 + c) * 128 : (p * 4 + c + 1) * 128],
                        xts[tti][:, c * TT : (c + 1) * TT],
                        start=(c == 0),
                        stop=(c == 3),
                    )
                nc.vector.tensor_scalar_add(
                    kt[:, p * S + t0 : p * S + t0 + TT], pk[:], bk_sb[:, p : p + 1]
                )

            def emit_q(tti, p):
                t0 = tti * TT
                pq = pa.tile([128, 512], F32, tag="pa")
                for c in range(4):
                    nc.tensor.matmul(
                        pq[:],
                        wq_sb[:, (p * 4 + c) * 128 : (p * 4 + c + 1) * 128],
                        qis[tti][:, c * TT : (c + 1) * TT],
                        start=(c == 0),
                        stop=(c == 3),
                    )
                nc.vector.tensor_scalar_add(
                    qt[:, p * S + t0 : p * S + t0 + TT], pq[:], bq_sb[:, p : p + 1]
                )

            def emit_y(qtile, ft, wide=False, act_copy=False):
                # output projection for one 128-token tile; `wide` borrows a
                # half-used logits-shaped psum so the tail pipelines deeper
                f0 = qtile * QTILE + ft * 128
                if wide:
                    pyw = lp.tile([128, QTILE], F32, tag="pl")
                    py = pyw[:, 0:512]
                else:
                    py = pa.tile([128, 512], F32, tag="pa")
                for p in range(2):
                    nc.tensor.matmul(
                        py[:],
                        outp[:, p * S + f0 : p * S + f0 + 128],
                        wo_sb[:, p * 512 : (p + 1) * 512],
                        start=(p == 0),
                        stop=(p == 1),
                    )
                yt = ypool.tile([128, 512], BF16, tag="yt")
                if act_copy:
                    nc.scalar.copy(yt[:], py[:])
                else:
                    nc.vector.tensor_copy(yt[:], py[:])
                nc.sync.dma_start(y[f0 : f0 + 128, :], yt[:])

            pending = []

            def emit_logits(h, qtile, kc):
                hh = h % 2
                p, base = h // 2, 64 * hh
                q0 = qtile * QTILE
                et = epool.tile([128, QTILE], BF16, tag="et")
                pl = lp.tile([128, QTILE], F32, tag="pl")
                for hf in range(2):
                    nc.tensor.matmul(
                        pl[:, hf * 512 : (hf + 1) * 512],
                        kt[base : base + 64, p * S + kc * KC : p * S + (kc + 1) * KC],
                        qt[
                            base : base + 64,
                            p * S + q0 + hf * 512 : p * S + q0 + (hf + 1) * 512,
                        ],
                        start=True,
                        stop=True,
                    )
                nc.scalar.activation(
                    et[:], pl[:], mybir.ActivationFunctionType.Exp
                )
                return et

            def emit_block_end(h, qtile, po, last=False):
                # evacuate po -> SBUF staging (frees the psum bank fast for
                # the next block); on the last block the reciprocal reads the
                # psum row directly instead so it starts 1.2us earlier
                p, hh = h // 2, h % 2
                ocol = p * S + qtile * QTILE
                rrow = rrpool.tile([VW, QTILE], F32R, tag="rrow")
                stg = spool.tile([VW, QTILE], F32, tag="stg")
                if last:
                    with nc.allow_low_precision(reason="f32r recip row"):
                        nc.vector.reciprocal(rrow[64:65, :], po[64:65, :])
                    nc.vector.tensor_copy(stg[:], po[:])
                else:
                    nc.vector.tensor_copy(stg[:], po[:])
                    with nc.allow_low_precision(reason="f32r recip row"):
                        nc.vector.reciprocal(rrow[64:65, :], stg[64:65, :])

                def finish(ocol=ocol, rrow=rrow, stg=stg, hh=hh):
                    # broadcast recip row across 64 partitions via a
                    # [1,64]-ones stationary matmul (standard (0,0) tile),
                    # then scale; odd heads reach partitions 64:128 via DMA
                    for hf in range(2):
                        rb = pa.tile([128, 512], F32, tag="pa")
                        nc.tensor.matmul(
                            rb[0:64, :],
                            ones_sb[64:65, :],
                            rrow[64:65, hf * 512 : (hf + 1) * 512],
                            start=True,
                            stop=True,
                        )
                        ssl = stg[0:64, hf * 512 : (hf + 1) * 512]
                        if hh == 0:
                            nc.vector.tensor_tensor(
                                outp[
                                    0:64,
                                    ocol + hf * 512 : ocol + (hf + 1) * 512,
                                ],
                                ssl,
                                rb[0:64, :],
                                op=mybir.AluOpType.mult,
                            )
                        else:
                            nc.vector.tensor_tensor(
                                ssl, ssl, rb[0:64, :], op=mybir.AluOpType.mult
                            )
                    if hh == 1:
                        nc.sync.dma_start(
                            outp[64:128, ocol : ocol + QTILE].bitcast(F32),
                            stg[0:64, :],
                        )

                pending.append(finish)

            def emit_attention_stream():
                # one continuous software pipeline over all (qt, h, kc)
                # steps: logits/exp of step i+1 are emitted before attnV of
                # step i, ACROSS block boundaries, so the ACT exp stream
                # never drains while a block finishes. qt1 ends on an even
                # head so the final outp write needs no partition-move DMA.
                steps = [
                    (h, qtile, kc)
                    for qtile in range(NQT)
                    for h in ((0, 1, 2, 3) if qtile == 0 else (0, 1, 3, 2))
                    for kc in range(NKC)
                ]
                po = None
                et_cur = emit_logits(*steps[0])
                for i, (h, qtile, kc) in enumerate(steps):
                    et_next = (
                        emit_logits(*steps[i + 1]) if i + 1 < len(steps) else None
                    )
                    # extra PE work goes before attnV so it never delays the
                    # next logits pair that paces the ACT exp stream
                    if kc == 2 and pending:
                        pending.pop(0)()
                    if kc in (3, 7, 11, 14) and spill:
                        spill.pop(0)()
                    if qtile == 1 and kc == 5:
                        emit_y(0, h * 2)
                    if qtile == 1 and kc == 11:
                        emit_y(0, h * 2 + 1)
                    if kc == 0:
                        po = op.tile([VW, QTILE], F32, tag="po")
                    for hf in range(2):
                        nc.tensor.matmul(
                            po[:, hf * 512 : (hf + 1) * 512],
                            vp[:, kc * VG + h * VW : kc * VG + (h + 1) * VW],
                            et_cur[:, hf * 512 : (hf + 1) * 512],
                            start=(kc == 0),
                            stop=(kc == NKC - 1),
                        )
                    et_cur = et_next
                    if kc == NKC - 1:
                        emit_block_end(h, qtile, po, last=(i == len(steps) - 1))

            # phase 1: everything attention blocks h0/h1 (pair 0) need,
            # DMAs ordered so the first V/K/Q matmuls unblock earliest
            emit_load_x(0)
            emit_early_weights()
            emit_load_q(0)
            nc.sync.dma_start(wq_sb[:], wq.rearrange("i p j -> p i j"))
            emit_load_x(1)
            emit_load_q(1)
            nc.sync.dma_start(wo_sb[:], wo.rearrange("i p j -> p i j"))
            emit_load_x(2)
            emit_load_x(3)
            emit_load_q(2)
            emit_load_q(3)
            for tti in range(NTT):
                emit_v(tti)
                if tti < 2:
                    emit_k(tti, 0)
                    emit_q(tti, 0)
            # phase 2: spilled into the early attention blocks' PE slack,
            # ordered by when the attention stream first needs each piece
            spill = [
                lambda: emit_k(2, 0),
                lambda: emit_k(3, 0),
                lambda: emit_q(0, 1),
                lambda: emit_k(0, 1),
                lambda: emit_k(1, 1),
                lambda: emit_q(1, 1),
                lambda: emit_k(2, 1),
                lambda: emit_k(3, 1),
                lambda: emit_q(2, 0),
                lambda: emit_q(3, 0),
                lambda: emit_q(2, 1),
                lambda: emit_q(3, 1),
            ]
            emit_attention_stream()
            assert not spill
            # keep the PE p-state warm through the final normalization
            # chain: idle gaps reset the ramp and would halve the speed of
            # the tail's output-projection matmuls
            warm = pa.tile([128, 512], F32, tag="pa")
            for _ in range(12):
                nc.tensor.matmul(
                    warm[0:64, 0:64],
                    ones_sb[64:65, :],
                    ones_sb[64:65, :],
                    start=True,
                    stop=True,
                )
            while pending:
                pending.pop(0)()
            # tail: alternate y psums between the pa ring and the now-idle
            # logits ring so the final tiles pipeline 4-deep; evac copies
            # ride the now-idle ACT engine and DMAs go out 2 tiles at a time
            for fp in range(4):
                yt2 = ypool.tile([128, 2 * 512], BF16, tag="yt2")
                for j in range(2):
                    ft = fp * 2 + j
                    f0 = (NQT - 1) * QTILE + ft * 128
                    if j == 0:
                        pyw = lp.tile([128, QTILE], F32, tag="pl")
                        py = pyw[:, 0:512]
                    else:
                        py = pa.tile([128, 512], F32, tag="pa")
                    for p in range(2):
                        nc.tensor.matmul(
                            py[:],
                            outp[:, p * S + f0 : p * S + f0 + 128],
                            wo_sb[:, p * 512 : (p + 1) * 512],
                            start=(p == 0),
                            stop=(p == 1),
                        )
                    nc.scalar.copy(yt2[:, j * 512 : (j + 1) * 512], py[:])
                f0 = (NQT - 1) * QTILE + fp * 256
                nc.sync.dma_start(
                    y[f0 : f0 + 256, :].rearrange("(j p) c -> p j c", p=128),
                    yt2[:],
                )

    _split_multi_waits(nc)
    return nc


_CACHE: dict = {}


def _prep_inputs(x, q, Wq, bq, Wk, bk, Wv, bv, Wo, bo):
    x = np.asarray(x, np.float32)
    q = np.asarray(q, np.float32)
    Wq, bq = np.asarray(Wq, np.float32), np.asarray(bq, np.float32)
    Wk, bk = np.asarray(Wk, np.float32), np.asarray(bk, np.float32)
    Wv, bv = np.asarray(Wv, np.float32), np.asarray(bv, np.float32)
    Wo = np.asarray(Wo, np.float32)

    scale = np.float32(1.0 / np.sqrt(np.float32(Dh)))
    Wqs, bqs = Wq * scale, bq * scale

    xTb = [
        np.ascontiguousarray(x[b].T.reshape(4, 128, S)).astype(NP_BF16)
        for b in range(B)
    ]
    qTb = [
        np.ascontiguousarray(q[b].T.reshape(4, 128, S)).astype(NP_BF16)
        for b in range(B)
    ]

    in_maps = []
    for c in range(NCORES):
        b, g = c // 2, c % 2
        hs = slice(g * 256, (g + 1) * 256)  # 4 heads' dims
        Wk_g, Wq_g, Wv_g = Wk[hs], Wqs[hs], Wv[hs]  # [256, 512]
        bk_g, bq_g, bv_g = bk[hs], bqs[hs], bv[hs]  # [256]
        # [pair, chunk, 128, 128]: chunk c of (pair rows).T
        wk_a = np.stack(
            [
                Wk_g[p * 128 : (p + 1) * 128].T.reshape(4, 128, 128)[ch]
                for p in range(2)
                for ch in range(4)
            ]
        )
        wq_a = np.stack(
            [
                Wq_g[p * 128 : (p + 1) * 128].T.reshape(4, 128, 128)[ch]
                for p in range(2)
                for ch in range(4)
            ]
        )
        wvT_a = np.ascontiguousarray(Wv_g.T.reshape(4, 128, 256))
        wo_a = np.stack(
            [
                np.ascontiguousarray(Wo[:, hs][:, p * 128 : (p + 1) * 128].T)
                for p in range(2)
            ]
        )
        bk_a = np.ascontiguousarray(bk_g.reshape(2, 128).T)
        bq_a = np.ascontiguousarray(bq_g.reshape(2, 128).T)
        bv_a = np.ascontiguousarray(np.tile(bv_g[None, :], (128, 2)))
        in_maps.append(
            {
                "xT": xTb[b],
                "qT": qTb[b],
                "wk": np.ascontiguousarray(wk_a).astype(NP_BF16),
                "wq": np.ascontiguousarray(wq_a).astype(NP_BF16),
                "wvT": np.ascontiguousarray(wvT_a).astype(NP_BF16),
                "wo": np.ascontiguousarray(wo_a, np.float32),
                "bk": np.ascontiguousarray(bk_a, np.float32),
                "bq": np.ascontiguousarray(bq_a, np.float32),
                "bv": np.ascontiguousarray(bv_a, np.float32),
            }
        )
    return in_maps


def kernel(x, q, Wq, bq, Wk, bk, Wv, bv, Wo, bo):
    _install_drain_patch()
    if "nc" not in _CACHE:
        _CACHE["nc"] = _build()
    nc = _CACHE["nc"]
    in_maps = _prep_inputs(x, q, Wq, bq, Wk, bk, Wv, bv, Wo, bo)
    res = run_bass_kernel_spmd(nc, in_maps, core_ids=list(range(NCORES)))
    bo64 = np.asarray(bo, np.float32).astype(np.float64)
    out = np.empty((B, S, D), np.float32)
    for b in range(B):
        acc = (
            res.results[2 * b]["y"].astype(np.float64)
            + res.results[2 * b + 1]["y"].astype(np.float64)
            + bo64
        )
        out[b] = acc.astype(np.float32)
    return out
